# revision 1
# baseline (speedup 1.0000x reference)
"""Gaussian-kernel weighted sum (retrieval_knn) on 8 Trainium2 NeuronCores.

    out[b] = sum_g w_g * exp(-||x_b - c_g||^2 / 2)

Strategy (data-parallel over the query batch, centers replicated):
  - Fold the whole exponent into one matmul via augmented features:
        t[b,g] = x_b . c_g  +  (ln|w_g| - ||c_g||^2/2)  +  (-||x_b||^2/2)
    The per-g term rides in as two extra contraction rows paired with ones
    on the query side; the per-b term is the ScalarE activation bias.
  - bf16 hi/lo split keeps the dot product at ~fp32 precision with two
    bf16 matmuls (full-rate on the PE) instead of one 4x-slower fp32 one:
        MM1 (K=128): [x_hi; x_lo] . [c_hi; c_hi]
        MM2 (K=66):  [x_hi; 1; 1] . [c_lo; a_hi; a_lo]
  - ScalarE computes exp with accum_out (sum along the free/g axis), so the
    16.8M-element exp never round-trips through another engine.  Signs of w
    are handled by sorting centers (positives first) and splitting the one
    ACT instruction that straddles the boundary; per-chunk partial sums are
    sign-combined by one tiny VectorE tensor_tensor_reduce per 128 queries.
"""

import numpy as np
import ml_dtypes

import concourse.bass as bass
import concourse.mybir as mybir
import concourse.tile as tile
from concourse import bacc
from concourse import bass_utils

BF16 = mybir.dt.bfloat16
F32 = mybir.dt.float32
NPBF16 = ml_dtypes.bfloat16

N_CORES = 8
B, G, D = 16384, 8192, 64
BL = B // N_CORES            # queries per core
NB = BL // 128               # 128-row query chunks per core
GT = 512                     # matmul tile width (one PSUM bank)
SUPER = 2048                 # PSUM half (4 banks) = one ACT chunk
NJ = G // SUPER              # g super-chunks
K2 = D + 2                   # contraction dim of MM2


def _segments(p_boundary):
    """ACT instruction layout: (j, offset, length, acc_slot, sign) per chunk,
    splitting the chunk that straddles the positive/negative boundary.
    Slots are dense so every acc column is written each b-chunk (no memset)."""
    segs = []
    slot = 0
    for j in range(NJ):
        lo, hi = j * SUPER, (j + 1) * SUPER
        if lo < p_boundary < hi:
            segs.append((j, lo, p_boundary - lo, slot, 1.0))
            slot += 1
            segs.append((j, p_boundary, hi - p_boundary, slot, -1.0))
            slot += 1
        else:
            segs.append((j, lo, SUPER, slot, 1.0 if hi <= p_boundary else -1.0))
            slot += 1
    return segs


def _build_unrolled(segs, rep):
    return _build(segs, rep=rep, unrolled=True)


def _build(segs, rep=1, unrolled=False):
    nc = bacc.Bacc(
        "TRN2",
        target_bir_lowering=False,
        debug=False,
        enable_asserts=False,
        num_devices=N_CORES,
    )

    d_r1 = nc.dram_tensor("r1", [128, G], BF16, kind="ExternalInput")
    d_r2 = nc.dram_tensor("r2", [K2, G], BF16, kind="ExternalInput")
    d_l1 = nc.dram_tensor("l1", [128, BL], BF16, kind="ExternalInput")
    d_l2 = nc.dram_tensor("l2", [K2, BL], BF16, kind="ExternalInput")
    d_qb = nc.dram_tensor("qb", [128, NB], F32, kind="ExternalInput")
    nseg = len(segs)
    d_sg = nc.dram_tensor("sg", [128, nseg], F32, kind="ExternalInput")
    d_out = nc.dram_tensor("out", [128, NB], F32, kind="ExternalOutput")

    with tile.TileContext(nc) as tc:
        from contextlib import ExitStack

        with ExitStack() as ctx:
            cpool = ctx.enter_context(tc.tile_pool(name="const", bufs=1))
            psum_pool = ctx.enter_context(
                tc.tile_pool(name="psum", bufs=2, space="PSUM")
            )
            apool = ctx.enter_context(tc.tile_pool(name="acc", bufs=2))

            r1 = cpool.tile([128, G], BF16)
            r2 = cpool.tile([K2, G], BF16)
            l1 = cpool.tile([128, BL], BF16)
            l2 = cpool.tile([K2, BL], BF16)
            qb = cpool.tile([128, NB], F32)
            sg = cpool.tile([128, nseg], F32)
            out_sb = cpool.tile([128, NB], F32)

            # Small operands first, then centers chunked so the first matmuls
            # can start before all of the centers have landed.
            nc.sync.dma_start(l1[:], d_l1.ap())
            nc.sync.dma_start(l2[:], d_l2.ap())
            nc.sync.dma_start(qb[:], d_qb.ap())
            nc.sync.dma_start(sg[:], d_sg.ap())
            for j in range(NJ):
                gs = slice(j * SUPER, (j + 1) * SUPER)
                nc.sync.dma_start(r1[:, gs], d_r1.ap()[:, gs])
                nc.sync.dma_start(r2[:, gs], d_r2.ap()[:, gs])

            segs_by_j = {}
            for s in segs:
                segs_by_j.setdefault(s[0], []).append(s)

            from contextlib import nullcontext

            if rep > 1 and unrolled:
                for _ in range(rep):
                    _emit_body(nc, tc, psum_pool, apool, segs_by_j,
                               r1, r2, l1, l2, qb, sg, out_sb)
            else:
                loop_ctx = (
                    tc.For_i(0, rep, 1, hint_engines=(mybir.EngineType.PE,))
                    if rep > 1
                    else nullcontext()
                )
                with loop_ctx:
                    _emit_body(nc, tc, psum_pool, apool, segs_by_j,
                               r1, r2, l1, l2, qb, sg, out_sb)
            nc.sync.dma_start(d_out.ap(), out_sb[:])

    nc.compile()
    return nc


def _emit_body(nc, tc, psum_pool, apool, segs_by_j, r1, r2, l1, l2, qb, sg, out_sb):
    nseg = sum(len(v) for v in segs_by_j.values())
    if True:
            for i in range(NB):
                bs = slice(i * 128, (i + 1) * 128)
                acc = apool.tile([128, nseg], F32)
                for j in range(NJ):
                    ps = psum_pool.tile([128, SUPER], F32)
                    # Same-stationary matmuls grouped so the PE does 2 weight
                    # switches per super-chunk instead of 8.
                    for k in range(SUPER // GT):
                        goff = j * SUPER + k * GT
                        nc.tensor.matmul(
                            ps[:, k * GT : (k + 1) * GT],
                            l1[:, bs],
                            r1[:, goff : goff + GT],
                            start=True,
                            stop=False,
                        )
                    for k in range(SUPER // GT):
                        goff = j * SUPER + k * GT
                        nc.tensor.matmul(
                            ps[:, k * GT : (k + 1) * GT],
                            l2[:, bs],
                            r2[:, goff : goff + GT],
                            start=False,
                            stop=True,
                        )
                    for (_, off, length, slot, _) in segs_by_j[j]:
                        loc = off - j * SUPER
                        # exp written in place over the PSUM inputs; only the
                        # per-instruction accum (sum along g) is consumed.
                        nc.scalar.activation(
                            ps[:, loc : loc + length],
                            ps[:, loc : loc + length],
                            mybir.ActivationFunctionType.Exp,
                            bias=qb[:, i : i + 1],
                            scale=1.0,
                            accum_out=acc[:, slot : slot + 1],
                        )
                ttr_out = apool.tile([128, nseg], F32, tag="ttr")
                nc.vector.tensor_mul(ttr_out[:], acc[:], sg[:])
                nc.vector.tensor_reduce(
                    out_sb[:, i : i + 1],
                    ttr_out[:],
                    mybir.AxisListType.X,
                    mybir.AluOpType.add,
                )


def _prep(input, inputs, weights):
    """Host-side preprocessing -> (shared in_map pieces, per-core pieces)."""
    x = np.asarray(input, dtype=np.float32)
    c = np.asarray(inputs, dtype=np.float32)
    w = np.asarray(weights, dtype=np.float32)

    # Sort centers: positive weights first.
    order = np.argsort(w < 0, kind="stable")
    c = c[order]
    w = w[order]
    p_boundary = int((w >= 0).sum())

    c64 = c.astype(np.float64)
    absw = np.abs(w.astype(np.float64))
    a = np.where(absw > 0, np.log(np.maximum(absw, 1e-300)), -1e4)
    a = (a - (c64 * c64).sum(1) / 2.0).astype(np.float32)
    a = np.maximum(a, np.float32(-1e4))

    def split(v):
        hi = v.astype(NPBF16).astype(np.float32)
        lo = (v - hi).astype(NPBF16)
        return hi.astype(NPBF16), lo

    c_hi, c_lo = split(c)
    a_hi, a_lo = split(a)

    r1 = np.empty((128, G), dtype=NPBF16)
    r1[0:64] = c_hi.T
    r1[64:128] = c_hi.T
    r2 = np.empty((K2, G), dtype=NPBF16)
    r2[0:64] = c_lo.T
    r2[64] = a_hi
    r2[65] = a_lo

    segs = _segments(p_boundary)
    sg = np.zeros((128, len(segs)), dtype=np.float32)
    for (_, _, _, slot, sign) in segs:
        sg[:, slot] = sign

    per_core = []
    for core in range(N_CORES):
        xs = x[core * BL : (core + 1) * BL]
        x_hi, x_lo = split(xs)
        l1 = np.empty((128, BL), dtype=NPBF16)
        l1[0:64] = x_hi.T
        l1[64:128] = x_lo.T
        l2 = np.empty((K2, BL), dtype=NPBF16)
        l2[0:64] = x_hi.T
        l2[64:66] = np.ones((2, BL), dtype=NPBF16)
        qb = (
            -(xs.astype(np.float64) ** 2).sum(1) / 2.0
        ).astype(np.float32).reshape(NB, 128).T.copy()
        per_core.append({"l1": l1, "l2": l2, "qb": qb})

    shared = {"r1": r1, "r2": r2, "sg": sg}
    return shared, per_core, segs


def kernel(input, inputs, weights):
    shared, per_core, segs = _prep(input, inputs, weights)
    nc = _build(segs)
    in_maps = [{**shared, **pc} for pc in per_core]
    res = bass_utils.run_bass_kernel_spmd(
        nc, in_maps, core_ids=list(range(N_CORES))
    )
    outs = []
    for r in res.results:
        o = r["out"]  # [128, NB]; query b = i*128 + p lives at o[p, i]
        outs.append(o.T.reshape(BL))
    return np.concatenate(outs).astype(np.float32)

